# revision 20
# baseline (speedup 1.0000x reference)
"""Multi-head attention Trainium2 kernel (nn_MultiHeadAttention_86423331930281).

Self-contained: builds a Bass/Tile SPMD kernel, data-parallel over batch
(B=8 -> one batch element per NeuronCore), runs on cores 0-7 via
run_bass_kernel_spmd, returns the full [8, 1024, 1024] f32 output.

Host-side prep (layout/cast only): q/k/v transposed to [d,s] and packed fp8
(e4m3) in DoubleRow k-pair layout, per-head-pair Q/K weights packed fp8 with
a x16 gain (compensated in the exp scale), Wv fp8 DR-packed with a x16 gain,
Wo fp8 packed per m-pair for a DoubleRow FC with a x16 gain, bias pre-scaled
x256 (both gains compensated by a 1/256 scale in the FC output copy).

Per-core algorithm (S=1024, D=1024, H=16 heads, E=64):
  - Q/K projections per head pair m as fp8 DoubleRow matmuls; m=0/m=1 run
    first so the PE starts ~2 DMAs in; the projection for m+2 is emitted
    inside m's attention loop.
  - V-projection as fp8 DoubleRow matmuls producing V1[t, h, e|1] with a
    trailing ones column per head; runs inside m=0's loop after 4 score-only
    steps so the PE covers the vT/wv input transfers.
  - attention per (m, j): scoresT[t,s] = KT_slice.T @ QT (K=64); exp split
    between ScalarE (spline exp) and DVE (bf16-bit-trick exp writing int16
    exponent bits; common-mode bias cancels in softmax).
  - PV in the [s, e] orientation: stationary = exp'd scoresT [t, s-block],
    moving = V1[t, e|1]; output attended[s-block, e] uses the full 128
    output partitions (half the PE columns of the [e, s] orientation).
    Denominators fall out as 1-col matmuls against the V1 ones column into
    a shared "misc" PSUM bank.
  - normalization: per-partition reciprocal [s,8sb] + one broadcast-AP
    tensor_tensor per (m, hh) (no DRAM round trip), written as
    attn[s, sb, hh, e] bf16; PE-transposed per (m, sb) into [he, s-block]
    bf16 PSUM slots (misc bank) and evacuated to fp8 attT2[mp][he, i, s]
    (m-pair DoubleRow layout for the FC).
  - FC: out[s, o] = attT2.T @ wo2 as 4 fp8-DoubleRow matmuls (K=256 each)
    plus a K=1 ones x bias-row matmul; epilogue is a scaled copy.
"""

import numpy as np
from contextlib import ExitStack

import concourse.bass as bass
import concourse.mybir as mybir
import concourse.tile as tile
from concourse.bass_utils import run_bass_kernel_spmd
from concourse.masks import make_identity

P = 128
S = 1024          # sequence length
DK = 1024         # qkv input dim
H = 16            # heads
E = 64            # per-head dim
HE = H * E        # 1024
OUT = 1024        # output dim
NT = S // P       # 8 s/t tiles
NK = DK // P      # 8 contraction tiles
NM = H // 2       # 8 head pairs
NMP = NM // 2     # 4 m-pairs (FC DoubleRow)
F32 = mybir.dt.float32
BF16 = mybir.dt.bfloat16
I16 = mybir.dt.int16
FP8 = mybir.dt.float8e4
AF = mybir.ActivationFunctionType
ALU = mybir.AluOpType
MMPM = mybir.MatmulPerfMode
SCALE = 1.0 / 32.0  # 1/sqrt(DK)

# Q/K weights are pre-scaled by WQK_GAIN on the host before fp8 rounding
# (pushes values out of the fp8 subnormal range); the resulting x256 gain
# on the scores is folded into the exp scale.
WQK_GAIN = 16.0
ESCALE = SCALE / (WQK_GAIN * WQK_GAIN)
# Wv / Wo fp8 gains (keep weights out of the fp8 subnormal range); both are
# compensated by FC_OSCALE in the FC output copy.  The bias is pre-scaled by
# 1/FC_OSCALE on the host so it comes out right.
WV_GAIN = 1.0
WO_GAIN = 1.0

# bf16-bit-trick exp on DVE: bits16 = round(x * EXPA + EXPB) reinterpreted
# as bf16 gives approx exp(x * ESCALE).  The additive constant only shifts a
# common-mode factor which softmax normalization cancels exactly.
LOG2E = 1.4426950408889634
EXPA = ESCALE * LOG2E * 128.0
EXPB = 16256.0 - 4.75


def _legalize_matmul_waits(nc):
    """This walrus build allows only ONE sync-wait command per instruction.
    Move all but the last wait of any multi-wait instruction onto freshly
    inserted nops immediately before it - same engine queue, so the
    blocking semantics are identical."""
    SKIP = ("NoOp", "Br", "Halt", "Sem", "Event")
    k = 0
    for f in nc.m.functions:
        for b in f.blocks:
            out = []
            for inst in b.instructions:
                si = getattr(inst, "sync_info", None)
                tname = type(inst).__name__
                if (not any(s in tname for s in SKIP) and si is not None
                        and si.on_wait and len(si.on_wait) > 1):
                    waits = list(si.on_wait)
                    for w in waits[:-1]:
                        nop = mybir.InstNoOp(
                            name=f"legalize-nop-{k}", ins=[], outs=[])
                        k += 1
                        nop.engine = inst.engine
                        nop.sync_info = mybir.SyncInfo(
                            on_wait=[w], on_update=[])
                        out.append(nop)
                    inst.sync_info = mybir.SyncInfo(
                        on_wait=[waits[-1]], on_update=list(si.on_update))
                out.append(inst)
            b.instructions[:] = out
    return k


def build(legalize=True, debug=False):
    nc = bass.Bass()
    dbg = {}
    if debug:
        dbg["qtm0"] = nc.dram_tensor("d_qtm0", (P, S), BF16, kind="ExternalOutput")
        dbg["ktm0"] = nc.dram_tensor("d_ktm0", (P, S), BF16, kind="ExternalOutput")
        dbg["v1_0"] = nc.dram_tensor("d_v1_0", (P, H, E + 1), BF16,
                                     kind="ExternalOutput")
        dbg["p0"] = nc.dram_tensor("d_p0", (P, S), BF16, kind="ExternalOutput")
        dbg["pa0"] = nc.dram_tensor("d_pa0", (P, 512), BF16, kind="ExternalOutput")
        dbg["pb0"] = nc.dram_tensor("d_pb0", (P, 512), I16, kind="ExternalOutput")
        dbg["den0"] = nc.dram_tensor("d_den0", (P, 16), F32, kind="ExternalOutput")
        dbg["attn0"] = nc.dram_tensor("d_attn0", (P, NT, 2, E), BF16,
                                      kind="ExternalOutput")
        dbg["attT2_0"] = nc.dram_tensor("d_attT2_0", (P, 2, S), BF16,
                                        kind="ExternalOutput")
    # q/k/v layout: [ki, jj, t, s] with d = jj*256 + t*128 + ki (DR k-pairs)
    qT_d = nc.dram_tensor("qT", (P, NK // 2, 2, S), FP8, kind="ExternalInput")
    kT_d = nc.dram_tensor("kT", (P, NK // 2, 2, S), FP8, kind="ExternalInput")
    vT_d = nc.dram_tensor("vT", (P, NK // 2, 2, S), BF16, kind="ExternalInput")
    # per head pair m: [ki, jj, t, he_pair]
    wq_d = nc.dram_tensor("wqp", (NM, P, NK // 2, 2, P), FP8,
                          kind="ExternalInput")
    wk_d = nc.dram_tensor("wkp", (NM, P, NK // 2, 2, P), FP8,
                          kind="ExternalInput")
    # [ki, jj, t, h*e] (DR k-pairs, x WV_GAIN)
    wv_d = nc.dram_tensor("wvp", (P, NK // 2, 2, HE), BF16,
                          kind="ExternalInput")
    # Wo.T per m-pair: [mp, he_within_m, i, out] (x WO_GAIN)
    wo_d = nc.dram_tensor("woT2", (NMP, P, 2, OUT), BF16, kind="ExternalInput")
    bo_d = nc.dram_tensor("boR", (1, OUT), BF16, kind="ExternalInput")
    out_d = nc.dram_tensor("out", (S, OUT), F32, kind="ExternalOutput")

    with tile.TileContext(nc) as tc, ExitStack() as ctx:
        const = ctx.enter_context(tc.tile_pool(name="const", bufs=1))
        xqk = ctx.enter_context(tc.tile_pool(name="xqk", bufs=1))
        v1p = ctx.enter_context(tc.tile_pool(name="v1p", bufs=NT))
        wo2p = ctx.enter_context(tc.tile_pool(name="wo2p", bufs=NMP))
        wslp = ctx.enter_context(tc.tile_pool(name="wslp", bufs=4))
        qkp = ctx.enter_context(tc.tile_pool(name="qkp", bufs=6))
        ptp = ctx.enter_context(tc.tile_pool(name="ptp", bufs=8))
        outp = ctx.enter_context(tc.tile_pool(name="outp", bufs=4))
        # PSUM budget (8 banks): 'ps' 2 banks (hh0 scores + qtm proj, ring1),
        # 'psa'/'psb' 1 bank each (ktm proj halves, hh1 score halves sca/scb
        # with single-engine readers, vproj halves), att ring 3x1 bank
        # (attended accumulators, (m+1,hh1) reuses (m,hh0)'s bank right
        # after m's hh0 normalize), misc 1 bank (denominators cols 0:32 +
        # three bf16 transpose slots).
        ps = ctx.enter_context(tc.tile_pool(name="ps", bufs=1, space="PSUM"))
        attps = ctx.enter_context(
            tc.tile_pool(name="attps", bufs=3, space="PSUM"))
        mps = ctx.enter_context(tc.tile_pool(name="mps", bufs=1, space="PSUM"))
        ph1 = ExitStack()
        vwp = ph1.enter_context(tc.tile_pool(name="vwp", bufs=1))

        ones_h = const.tile([P, H], BF16, name="ones_h")
        nc.gpsimd.memset(ones_h[:], 1.0)
        ones_fc = const.tile([1, P], BF16, name="ones_fc")
        nc.gpsimd.memset(ones_fc[:], 1.0)
        ident = const.tile([P, P], BF16, name="ident")
        make_identity(nc, ident[:])
        bo_row = const.tile([1, OUT], BF16, name="bo_row")
        nc.gpsimd.dma_start(bo_row[:], bo_d[:, :])

        misc = mps.tile([P, 512], F32, name="misc")

        def den_ap(m, hh):
            return misc[:, (m % 2) * 16 + hh * 8:(m % 2) * 16 + hh * 8 + 8]

        # ---- input DMAs.  SP queue: wq/wk then qT + wv; Pool queue: kT, vT,
        # woT2, bo.  proj m=0 starts as soon as wq0 + qT0 land.
        def load_wqk(m, queue=None):
            q_ = queue or nc.sync
            wq = wslp.tile([P, NK // 2, 2, P], FP8, name=f"wq{m}", tag="wsl")
            wk = wslp.tile([P, NK // 2, 2, P], FP8, name=f"wk{m}", tag="wsl")
            q_.dma_start(wq[:], wq_d[m])
            q_.dma_start(wk[:], wk_d[m])
            return wq, wk

        # DMA_ENGINES is a serialized device in the timeline model and its
        # service order follows issue order, so ALL input loads go on the SP
        # queue in strict priority order: m0/m1 projection inputs first, then
        # V-side bulk, then the FC weights.
        qTt = xqk.tile([P, NK // 2, 2, S], FP8, name="qTt", tag="qT")
        kTt = xqk.tile([P, NK // 2, 2, S], FP8, name="kTt", tag="kT")
        vTt = vwp.tile([P, NK // 2, 2, S], BF16, name="vTt", tag="vT")
        wvt = vwp.tile([P, NK // 2, 2, HE], BF16, name="wvt", tag="wv")
        wqk = [None, None]
        wqk[0] = load_wqk(0)
        for c in range(NK // 2):
            nc.sync.dma_start(qTt[:, c, :, :], qT_d[:, c, :, :])
            nc.sync.dma_start(kTt[:, c, :, :], kT_d[:, c, :, :])
        wqk[1] = load_wqk(1)
        for c in range(NK // 2):
            nc.sync.dma_start(wvt[:, c, :, :], wv_d[:, c, :, :])
            nc.sync.dma_start(vTt[:, c, :, :], vT_d[:, c, :, :])

        wo2_t = []
        for mp_ in range(NMP):
            t = wo2p.tile([P, 2, OUT], BF16, name=f"wo2_{mp_}", tag="wo2")
            nc.sync.dma_start(t[:], wo_d[mp_])
            wo2_t.append(t)

        def proj_mms(wm, xt, pst, c0, c1):
            for sh in range(c0 // 512, c1 // 512):
                o0 = sh * 512 - c0
                for jj in range(NK // 2):
                    nc.tensor.matmul(
                        pst[:, o0:o0 + 512],
                        wm[:, jj, :, :],
                        xt[:, jj, :, sh * 512:(sh + 1) * 512],
                        start=(jj == 0),
                        stop=(jj == NK // 2 - 1),
                        perf_mode=MMPM.DoubleRow)

        def proj_qk(m):
            """QT_m/KT_m [he_pair=128, s]; evacuated as bf16 per 512-col
            half so dependent scores can start on subtile waits."""
            wqm, wkm = wqk[m % 2]
            qkm = []
            for wi, (wm, xt, nm) in enumerate(
                    ((wqm, qTt, "qtm"), (wkm, kTt, "ktm"))):
                if wi == 0:
                    psts = [(ps.tile([P, S], F32, tag="ps",
                                     name=f"{nm}ps{m}"), 0, S)]
                else:
                    psts = [(ps.tile([P, 512], F32, tag="psa",
                                     name=f"{nm}psa{m}"), 0, 512),
                            (ps.tile([P, 512], F32, tag="psb",
                                     name=f"{nm}psb{m}"), 512, S)]
                t = qkp.tile([P, S], BF16, tag="qt", name=f"{nm}{m}")
                for pi, (pst, c0, c1) in enumerate(psts):
                    proj_mms(wm, xt, pst, c0, c1)
                    # evacuate in halves on ACT + DVE in parallel (this
                    # chain gates the next pair's score banks)
                    if c1 - c0 == S:
                        nc.scalar.copy(t[:, 0:512], pst[:, 0:512])
                        nc.vector.tensor_copy(t[:, 512:S], pst[:, 512:S])
                    elif pi == 0:
                        nc.scalar.copy(t[:, c0:c1], pst[:, 0:c1 - c0])
                    else:
                        nc.vector.tensor_copy(t[:, c0:c1], pst[:, 0:c1 - c0])
                qkm.append(t)
            if m + 2 < NM:
                wqk[m % 2] = load_wqk(m + 2)
            return qkm

        # ---- phase A: Q/K proj for m=0,1 (starts the PE early; m=1 covers
        # the tail of the vT/wv input stream)
        qtm_next = proj_qk(0)
        qtm_next2 = proj_qk(1)
        if debug:
            nc.sync.dma_start(dbg["qtm0"][:, :], qtm_next[0][:])
            nc.sync.dma_start(dbg["ktm0"][:, :], qtm_next[1][:])

        # ---- phase B: V projection (+ ones column), fp8 DoubleRow;
        # alternating 1-bank accumulators so the v1 evacuation overlaps the
        # next half's matmuls
        v1_t = []

        def vproj_tile(i):
            v1 = v1p.tile([P, H, E + 1], BF16, tag="v1", name=f"v1_{i}")
            nc.vector.tensor_copy(v1[:, :, E], ones_h[:])
            for nh, tg in ((0, "psa"), (1, "psb")):
                pst = ps.tile([P, 512], F32, tag=tg, name=f"vp{i}_{nh}")
                for jj in range(NK // 2):
                    for tt in range(2):
                        nc.tensor.matmul(
                            pst[:],
                            vTt[:, jj, tt, i * P:(i + 1) * P],
                            wvt[:, jj, tt, nh * 512:(nh + 1) * 512],
                            start=(jj == 0 and tt == 0),
                            stop=(jj == NK // 2 - 1 and tt == 1))
                nc.vector.tensor_copy(
                    v1[:, nh * 8:(nh + 1) * 8, 0:E],
                    pst[:].rearrange("p (h e) -> p h e", e=E))
            v1_t.append(v1)

        # ---- phase C: attention m loop
        att_prev = {}     # m -> (att_ps_hh0, att_ps_hh1)
        attn_t = {}       # m -> normalized attn sbuf tile [P, 8sb, 2hh, E]
        attT2 = []        # mp -> [P, 2, S] fp8
        late = {}

        def norm_hh(pm, hh):
            nc.vector.tensor_tensor(
                attn_t[pm][:, :, hh, :],
                att_prev[pm][hh][:].rearrange("p (g e) -> p g e", e=E),
                late[f"rcp{pm}"][:, hh].to_broadcast((P, NT, E)),
                ALU.mult)

        def start_norm(pm):
            rcp = late["recp"].tile([P, 2, NT, 1], F32, tag="rcp",
                                    name=f"rcp{pm}", bufs=2)
            late[f"rcp{pm}"] = rcp
            attn_t[pm] = late["attnp"].tile([P, NT, 2, E], BF16, tag="attn",
                                            name=f"attn{pm}", bufs=2)
            if debug and pm == 0:
                dent = const.tile([P, 16], F32, name="dent")
                nc.vector.tensor_copy(dent[:], misc[:, 0:16])
                nc.sync.dma_start(dbg["den0"][:, :], dent[:])
            for hh in range(2):
                nc.vector.reciprocal(rcp[:, hh, :, 0], den_ap(pm, hh))
            norm_hh(pm, 0)

        def tslot(pm, sb):
            # transpose scratch: the (pm, hh1) att bank is fully consumed by
            # the normalize before transposes start and is not re-allocated
            # until (pm+2, hh0); the m7 tail instead borrows the den bank
            # (cols 32..480 + wrap) since all den accumulation is done then.
            if pm < NM - 1:
                return att_prev[pm][1][:, 64 * sb:64 * sb + 64].bitcast(BF16)
            c = 32 + 64 * (sb % 7)
            return misc[:, c:c + 64].bitcast(BF16)

        def transp(pm, sb):
            nc.tensor.transpose(tslot(pm, sb), attn_t[pm][:, sb], ident[:])

        def evac_attT(pm, sb, eng):
            eng(attT2[pm // 2][:, pm % 2, sb * P:(sb + 1) * P], tslot(pm, sb))

        pend = []
        pops = [0]

        def pv_step(m_, j, hh, pparts, half):
            # PSUM zero-region semantics: start=True marks the WHOLE 2KB
            # bank pending-zero, so exactly ONE matmul per bank lifetime may
            # carry start=True (the first); every other group's first write
            # consumes its own pending range (= overwrite) and later writes
            # accumulate.
            att_ps = att_prev[m_][hh]
            h = 2 * m_ + hh
            for sbk in range(half * 4, half * 4 + 4):
                pt = pparts[sbk // 4][:, (sbk % 4) * P:(sbk % 4 + 1) * P]
                nc.tensor.matmul(
                    att_ps[:, sbk * E:(sbk + 1) * E],
                    pt, v1_t[j][:, h, 0:E],
                    start=(j == 0 and sbk == 0),
                    stop=(j == NT - 1 and sbk == 7),
                    skip_group_check=True)
                nc.tensor.matmul(
                    den_ap(m_, hh)[:, sbk:sbk + 1],
                    pt, v1_t[j][:, h, E:E + 1],
                    start=(j == 0 and hh == 1 and sbk == 0),
                    stop=(j == NT - 1 and hh == 0 and sbk == 7),
                    skip_group_check=True)

        def pop_one():
            ent = pend[0][pops[0] // 2]
            pv_step(*ent, pops[0] % 2)
            pops[0] += 1
            if pops[0] == 4:
                pend.pop(0)
                pops[0] = 0

        for m in range(NM):
            qtm, ktm = qtm_next
            if m + 1 < NM:
                qtm_next = qtm_next2

            att_prev[m] = tuple(
                attps.tile([P, 512], F32, tag="att", name=f"att{m}_{hh}")
                for hh in range(2))


            # software-pipelined: PV lags scores by TWO steps (four during
            # m0's input-transfer window), so the exp chain latency never
            # stalls the PE.  The backlog carries across m boundaries.
            LAG = 6 if m == 0 else 3
            for j in range(NT):
                # m0: the bulk vT/wv loads stream during the first score
                # steps; the V projection is spread over j3..j6 just behind
                # them (PV pops need v1_t[j] from j4 on), and wo2 loads last
                if m == 0 and 5 <= j <= 7:
                    vproj_tile(2 * (j - 5))
                    vproj_tile(2 * (j - 5) + 1)
                if m == 1 and j == 0:
                    vproj_tile(6)
                    vproj_tile(7)
                    if debug:
                        nc.sync.dma_start(dbg["v1_0"][:, :, :], v1_t[0][:])
                    ph1.close()
                    late["attnp"] = ctx.enter_context(
                        tc.tile_pool(name="attnp", bufs=2))
                    late["recp"] = ctx.enter_context(
                        tc.tile_pool(name="recp", bufs=2))
                    attTp = ctx.enter_context(
                        tc.tile_pool(name="attTp", bufs=NMP))
                    for mp_ in range(NMP):
                        attT2.append(attTp.tile([P, 2, S], BF16,
                                                name=f"attT2_{mp_}"))
                # previous pair's normalize / transpose / evac chain
                # pops first: PE chews PV sub-steps while the previous
                # step's exps drain the score banks; then hh0 (big 'ps'
                # tile, one ACT exp) before hh1 (psa/psb halves, ACT+DVE).
                cur = []
                npop = 0
                while (pend and npop < 4
                       and (len(pend) > LAG
                            or (len(pend) == LAG and npop < 2))):
                    pop_one()
                    npop += 1
                # normalize chain after the pops: with LAG=3, (pm, j7) fully
                # retires in j==3's pops, and the first (m, hh1) ring-slot
                # write (which must follow norm-hh0(pm)) happens at j==4.
                if m >= 1:
                    pm = m - 1
                    if j == 3:
                        start_norm(pm)
                    elif j == 4:
                        norm_hh(pm, 1)
                        if debug and pm == 0:
                            nc.sync.dma_start(dbg["attn0"][:, :, :, :],
                                              attn_t[0][:])
                    elif j == 5:
                        for sb in range(4):
                            transp(pm, sb)
                        evac_attT(pm, 0, nc.scalar.copy)
                        evac_attT(pm, 1, nc.vector.tensor_copy)
                    elif j == 6:
                        for sb in range(4, 8):
                            transp(pm, sb)
                        evac_attT(pm, 2, nc.scalar.copy)
                        evac_attT(pm, 3, nc.vector.tensor_copy)
                    elif j == 7:
                        evac_attT(pm, 4, nc.scalar.copy)
                        evac_attT(pm, 5, nc.vector.tensor_copy)
                        evac_attT(pm, 6, nc.scalar.copy)
                        evac_attT(pm, 7, nc.vector.tensor_copy)
                for hh in (1, 0):
                    hs = slice(hh * E, (hh + 1) * E)
                    if hh == 0:
                        sc = ps.tile([P, S], F32, tag="ps",
                                     name=f"sc{m}_{j}")
                        for sh in range(2):
                            nc.tensor.matmul(
                                sc[:, sh * 512:(sh + 1) * 512],
                                ktm[hs, j * P:(j + 1) * P],
                                qtm[hs, sh * 512:(sh + 1) * 512],
                                start=True, stop=True)
                        ptile = ptp.tile([P, S], BF16, tag="pt",
                                         name=f"p{m}_{j}", bufs=8)
                        nc.scalar.activation(ptile[:], sc[:],
                                             AF.Exp, scale=ESCALE)
                        if debug and m == 0 and j == 0:
                            nc.sync.dma_start(dbg["p0"][:, :], ptile[:])
                        cur.append((m, j, hh, (ptile[:, 0:512],
                                               ptile[:, 512:S])))
                    else:
                        sca = ps.tile([P, 512], F32, tag="psa",
                                      name=f"sca{m}_{j}")
                        scb = ps.tile([P, 512], F32, tag="psb",
                                      name=f"scb{m}_{j}")
                        for sh, sct in ((0, sca), (1, scb)):
                            nc.tensor.matmul(
                                sct[:],
                                ktm[hs, j * P:(j + 1) * P],
                                qtm[hs, sh * 512:(sh + 1) * 512],
                                start=True, stop=True)
                        pa = ptp.tile([P, 512], BF16, tag="pta",
                                      name=f"pa{m}_{j}", bufs=8)
                        pb = ptp.tile([P, 512], I16, tag="ptb",
                                      name=f"pb{m}_{j}", bufs=8)
                        nc.scalar.activation(pa[:], sca[:],
                                             AF.Exp, scale=ESCALE)
                        nc.vector.tensor_scalar(
                            pb[:], scb[:],
                            EXPA, EXPB, ALU.mult, ALU.add)
                        if debug and m == 0 and j == 0:
                            nc.sync.dma_start(dbg["pa0"][:, :], pa[:])
                            nc.sync.dma_start(dbg["pb0"][:, :], pb[:])
                        cur.append((m, j, hh, (pa[:], pb.bitcast(BF16)[:])))
                # Q/K proj of m+2 after the last scores: PE fills the final
                # exp latencies; its evacuations overlap the PV drain
                if j == NT - 1 and m + 2 < NM:
                    qtm_next2 = proj_qk(m + 2)
                pend.append(cur)
            # drain down to the steady two-step backlog (fully at the end)
            keep = 0 if m == NM - 1 else 3
            while pend and (len(pend) > keep or pops[0] > 0):
                pop_one()
                if len(pend) == keep and pops[0] == 0:
                    break

        # ---- m=7 tail: normalize + transpose + evac, FC per s-block as
        # soon as its attT column block lands
        m7 = NM - 1
        start_norm(m7)
        norm_hh(m7, 1)
        for sb in range(4):
            transp(m7, sb)

        # ---- phase D: FC  out[st, oh] = sum_mp attT2[mp].T @ wo2[mp] (DR)
        #              + ones.T @ bo_row   (bias), epilogue: scaled copy
        def fc_tile(st, oh, ei):
            pso = attps.tile([P, 512], F32, tag="att", name=f"fc{st}_{oh}")
            for mp_ in range(NMP):
                for i_ in range(2):
                    nc.tensor.matmul(
                        pso[:],
                        attT2[mp_][:, i_, st * P:(st + 1) * P],
                        wo2_t[mp_][:, i_, oh * 512:(oh + 1) * 512],
                        start=(mp_ == 0 and i_ == 0), stop=False)
            nc.tensor.matmul(
                pso[:], ones_fc[:], bo_row[:, oh * 512:(oh + 1) * 512],
                start=False, stop=True)
            ot = outp.tile([P, 512], F32, tag="out", name=f"out{st}_{oh}")
            if ei == 0:
                nc.scalar.copy(ot[:], pso[:])
            else:
                nc.vector.tensor_copy(ot[:], pso[:])
            (nc.sync if ei == 0 else nc.gpsimd).dma_start(
                out_d[st * P:(st + 1) * P, oh * 512:(oh + 1) * 512], ot[:])

        if debug:
            nc.sync.dma_start(dbg["attT2_0"][:, :, :], attT2[0][:])
        for sb in range(NT):
            if sb >= 4:
                transp(m7, sb)
            # tail: ACT is free (no more exps) - split for latency
            evac_attT(m7, sb,
                      nc.scalar.copy if sb % 2 == 0 else nc.vector.tensor_copy)
            fc_tile(sb, 0, sb % 2)
            fc_tile(sb, 1, 1 - sb % 2)
    if legalize:
        _legalize_matmul_waits(nc)
    return nc


_NC_CACHE = {}


def _get_nc():
    if "nc" not in _NC_CACHE:
        _NC_CACHE["nc"] = build()
    return _NC_CACHE["nc"]


def _host_pack(query, key, value, Wq, Wk, Wv, Wo, bo):
    """Per-problem host-side layout prep (transpose + cast only)."""
    bf16 = mybir.dt.np(BF16)
    fp8 = mybir.dt.np(FP8)

    # q/k/v: [s, d] -> [ki, jj, t, s] with d = jj*256 + t*128 + ki
    def packx(x, dt):
        t = x.transpose(0, 2, 1).reshape(-1, NK // 2, 2, P, S)
        return np.ascontiguousarray(t.transpose(0, 3, 1, 2, 4)).astype(dt)

    qT = packx(query, fp8)
    kT = packx(key, fp8)
    vT = packx(value, bf16)

    # Wq [h, d, e] with d = jj*256 + t*128 + ki, h = 2m + hh ->
    # [m, ki, jj, t, (hh e)]
    def packw(W, gain):
        t = (W * gain).reshape(NM, 2, NK // 2, 2, P, E)
        t = t.transpose(0, 4, 2, 3, 1, 5)
        return np.ascontiguousarray(
            t.reshape(NM, P, NK // 2, 2, P)).astype(fp8)

    wqp = packw(Wq, WQK_GAIN)
    wkp = packw(Wk, WQK_GAIN)
    # Wv [h, d, e] -> [ki, jj, t, (h e)] with d = jj*256 + t*128 + ki
    wvp = np.ascontiguousarray(
        Wv.reshape(H, NK // 2, 2, P, E)
        .transpose(3, 1, 2, 0, 4).reshape(P, NK // 2, 2, HE)).astype(bf16)
    # Wo [out, he] -> [mp, p(he within m), i(m half), out]
    woT2 = np.ascontiguousarray(
        Wo.T.reshape(NMP, 2, P, OUT).transpose(0, 2, 1, 3)
    ).astype(bf16)
    boR = bo[None, :].astype(bf16)
    return qT, kT, vT, wqp, wkp, wvp, woT2, boR


def kernel(query, key, value, Wq, Wk, Wv, Wo, bo, **run_kwargs):
    query = np.asarray(query, dtype=np.float32)
    key = np.asarray(key, dtype=np.float32)
    value = np.asarray(value, dtype=np.float32)
    Wq = np.asarray(Wq, dtype=np.float32)
    Wk = np.asarray(Wk, dtype=np.float32)
    Wv = np.asarray(Wv, dtype=np.float32)
    Wo = np.asarray(Wo, dtype=np.float32)
    bo = np.asarray(bo, dtype=np.float32)
    B = query.shape[0]
    assert B == 8, f"expected batch 8, got {B}"

    qT, kT, vT, wqp, wkp, wvp, woT2, boR = _host_pack(
        query, key, value, Wq, Wk, Wv, Wo, bo)

    nc = _get_nc()
    in_maps = []
    for b in range(B):
        in_maps.append({
            "qT": qT[b], "kT": kT[b], "vT": vT[b],
            "wqp": wqp, "wkp": wkp, "wvp": wvp, "woT2": woT2, "boR": boR,
        })
    res = run_bass_kernel_spmd(nc, in_maps, core_ids=list(range(B)),
                               **run_kwargs)
    out = np.stack([r["out"] for r in res.results], axis=0)
    if run_kwargs.get("trace"):
        _NC_CACHE["last_result"] = res
    return out
